# revision 35
# baseline (speedup 1.0000x reference)
"""COIL-style sparse-attention scoring kernel for Trainium2 (8 NeuronCores).

Reference computation:
    scores[q,i,d,j] = <query_tok_embs[q,i], doc_tok_embs[d,j]>         (K=32)
    masked = where(query_ids[q,i]==doc_ids[d,j], scores, 0)
    tok    = masked.max(axis=j)                                        (192 -> 1)
    tok_scores[q,d] = sum_i w[q,i] * tok[q,i,d]    (w drops CLS + SEP)
    out = tok_scores + query_cls_emb @ doc_cls_emb.T

Device strategy: data-parallel over the 64 queries (8 per core).  COIL is an
inverted-list model -- a doc position (d,j) can only contribute to a query
token with the *same* token id.  With |vocab|=5000 and 256 query tokens per
core, only ~5% of the 24576 doc positions match ANY of the core's query
tokens.  The host (integer id bookkeeping only -- all float math stays on
device) keeps just those columns, padded to a small per-doc slot budget
tiered by match count (docs sorted descending; see PLANS), shrinking the
score matmul from 24576 columns to ~900 per 128-row block.

Exact-match masking folds into the contraction via a 13-bit +/-1 id code:
  aug[i,col] = <q_i, d_col>                      (rows 0:32,  fp16 single term)
             + C * sum_b qbit[i,b]*dbit[col,b]   (rows 32:45, q side +/-C, doc +/-1)
             - 13*C                              (row 45: q side 1, doc -13C)
full id match    -> aug = score (code dot = +13C cancels the offset row)
id mismatch     -> aug <= score - 2C < 0   (C > max|score|, bounded on host)
padding columns -> aug = 0 (all-zero column)
so tok[i,d] = relu(max over the doc's slot columns) reproduces the reference
exactly: the reference max always sees a 0 from a non-matched where() zero,
because per-doc matches (<= 32) never cover all 192 positions.

The "quad" plan runs score matmuls in two 64-row PE quadrants (K=46 rows
duplicated at SBUF partitions 0:46 and 64:110, each quadrant owning half
the docs), which halves the bytes per SBUF partition row of the main blob
-- DMA completion time here is governed by bytes-per-row, not total bytes.
Per core: 12 fp16 score matmuls in 8 PSUM waves, DVE segmented max into
tok scores, ACT relu (fp16 out), CLS scores as 6 single-term fp16 matmuls
+ per-token weighted sums folded into the same [8,128] PSUM accumulation,
one output DMA.  All input DMAs ride the two hardware DGE queues
(sync/scalar) as column-chunks issued in consumption order; the software
gpsimd queue is never on the critical path.
"""

import numpy as np
from contextlib import ExitStack

import concourse.bass as bass
import concourse.bacc as bacc
import concourse.mybir as mybir
import concourse.tile as tile
from concourse.bass_utils import run_bass_kernel_spmd

F32 = mybir.dt.float32
F16 = mybir.dt.float16

# problem shape (hardcoded per contract)
BQ, LQ, BD, LD, TOK_D, CLS_D = 64, 32, 128, 192, 32, 768
NCORES = 8
QPC = BQ // NCORES          # 8 queries per core
NBLK = 2                    # two row-blocks of 128 = 4 queries x 32 tokens
ROWS = 128
NBITS = 13                  # 2^13 = 8192 > 5000 vocab
KC = TOK_D + NBITS + 1      # 46 = embs + id code + offset row
TN = 512                    # cols per matmul = one full PSUM bank


# wave plans: list of waves; each wave = list of (ncols, ndocs, cap) segments.
# docs are laid out in plan order (descending match count); tokred column
# offsets are implied.  Plans are tried in order; first whose slot
# capacities fit the core's match pattern wins.
#
# "quad" runs the score matmuls in two 64-row PE quadrants: the K=46
# contraction rows are duplicated at SBUF partitions 0:46 and 64:110, and
# each quadrant owns half the docs (alternating by rank).  That halves the
# bytes per SBUF partition row of the A blob, and DMA completion time is
# governed by bytes-per-row.  Group layout (per quadrant, per block):
# 16 docs @cap24 + 48 @cap12 = 960 columns, as an a-wave (1 PSUM bank,
# ready as soon as the first half of A lands) and a b-wave (2 banks).
QUAD_GROUP = [[(384, 16, 24)], [(576, 48, 12)]]
PLANS = {
    "quad": None,   # handled by the quadrant path; fit-checked like hetero2
    "hetero2": [[(256, 8, 32), (384, 24, 16)], [(768, 64, 12)], [(384, 32, 12)]],
    "hetero": [[(512, 16, 32), (512, 32, 16)], [(1024, 64, 16)], [(256, 16, 16)]],
    "uniform24": [[(1536, 64, 24)], [(1536, 64, 24)]],
    "dense": [[(1536, 8, 192)] for _ in range(16)],
}


def _plan_caps(plan_name):
    """per-rank slot capacity list (docs in descending match count)."""
    if plan_name == "quad":
        gcaps = []
        for ncols, ndocs, cap in (s for segs in QUAD_GROUP for s in segs):
            gcaps += [cap] * ndocs
        return [gcaps[r // 2] for r in range(BD)]
    caps = []
    for ncols, ndocs, cap in (s for segs in PLANS[plan_name] for s in segs):
        caps += [cap] * ndocs
    return caps


def _plan_fits(plan_name, pd_sorted):
    caps = _plan_caps(plan_name)
    return all(int(pd_sorted[r]) <= caps[r] for r in range(BD))


def build_nc_quad():
    gcols = sum(s[0] for s in sum(QUAD_GROUP, []))      # 896 per quadrant
    atot = NBLK * ROWS + gcols                          # 1152
    nc = bacc.Bacc(
        "TRN2",
        target_bir_lowering=False,
        debug=False,
        num_devices=NCORES,
    )
    a_d = nc.dram_tensor("a", [110, atot], F16, kind="ExternalInput")
    b_d = nc.dram_tensor("b", [128, 48 + CLS_D], F16, kind="ExternalInput")
    c_d = nc.dram_tensor("c", [128, NBLK * QPC], F16, kind="ExternalInput")
    out_d = nc.dram_tensor("out", [QPC, BD], F32, kind="ExternalOutput")

    rhs0 = NBLK * ROWS

    with tile.TileContext(nc) as tc, ExitStack() as ctx:
        const = ctx.enter_context(tc.tile_pool(name="const", bufs=1))
        # a-waves (320 cols = 1 bank) and b-waves (576 cols = 2 banks) get
        # separate pools: 3*1 + 2*2 + 1 output bank = 8 PSUM banks exactly
        pa = ctx.enter_context(tc.tile_pool(name="pa", bufs=3, space="PSUM"))
        pb = ctx.enter_context(tc.tile_pool(name="pb", bufs=2, space="PSUM"))
        opsum = ctx.enter_context(tc.tile_pool(name="opsum", bufs=1, space="PSUM"))
        work = ctx.enter_context(tc.tile_pool(name="work", bufs=1))

        a_t = const.tile([110, atot], F16, tag="a")
        b_t = const.tile([128, 48 + CLS_D], F16, tag="b")
        c_t = const.tile([128, NBLK * QPC], F16, tag="c")

        engs = [nc.sync, nc.scalar]
        awend = rhs0 + sum(s[0] for s in QUAD_GROUP[0])   # a-wave data end
        edges = [0, awend // 2, awend, (awend + atot) // 2, atot]
        for i in range(4):
            c0, c1 = edges[i], edges[i + 1]
            engs[i % 2].dma_start(a_t[:, c0:c1], a_d[:, c0:c1])
        nc.sync.dma_start(b_t[:, 0:408], b_d[:, 0:408])
        nc.scalar.dma_start(b_t[:, 408:], b_d[:, 408:])
        nc.scalar.dma_start(c_t[:], c_d[:])

        out_ps = opsum.tile([QPC, BD], F32, tag="out_ps")

        tokreds = []
        for bi in range(NBLK):
            tokreds.append(
                work.tile([ROWS, BD], F32, tag=f"tokred{bi}", name=f"tokred{bi}")
            )
        # all four a-waves first: they only need the first half of A, and
        # fill the PE while the second half is still in flight
        seg_off = [rhs0]
        for segs in QUAD_GROUP:
            seg_off.append(seg_off[-1] + sum(s[0] for s in segs))
        with tc.high_priority():
            for kind, segs in enumerate(QUAD_GROUP):
                for bi in range(NBLK):
                    for g in range(2):
                        pool = pa if kind == 0 else pb
                        p0 = 64 * g
                        lhs = a_t[p0:p0 + KC, bi * ROWS:(bi + 1) * ROWS]
                        wcols = sum(s[0] for s in segs)
                        ps = pool.tile([128, wcols], F32, tag=f"score{kind}",
                                       name=f"ps{kind}_{bi}_{g}")
                        for k in range(0, wcols, TN):
                            n = min(TN, wcols - k)
                            nc.tensor.matmul(
                                ps[:, k:k + n],
                                lhs,
                                a_t[p0:p0 + KC,
                                    seg_off[kind] + k:seg_off[kind] + k + n],
                                start=True, stop=True,
                            )
                        off = 0
                        slot = 64 * g + sum(
                            s[1] for ss in QUAD_GROUP[:kind] for s in ss
                        )
                        for ncols, ndocs, cap in segs:
                            nc.vector.reduce_max(
                                tokreds[bi][:, slot:slot + ndocs],
                                ps[:, off:off + ncols].rearrange(
                                    "p (d s) -> p d s", s=cap
                                ),
                                axis=mybir.AxisListType.X,
                            )
                            off += ncols
                            slot += ndocs

        for k in range(6):
            nc.tensor.matmul(
                out_ps[:],
                b_t[:, k * QPC:(k + 1) * QPC],
                b_t[:, 48 + k * 128:48 + (k + 1) * 128],
                start=(k == 0), stop=False,
            )

        # relu + weighted sum per (block, quadrant) piece: each [*, 64]
        # column region finishes as soon as its reduces land.  fp16 dec and
        # sel weights (exact 0/1) make these 1-cycle/col matmuls.
        for bi in range(NBLK):
            dec = work.tile([ROWS, BD], F16, tag=f"tokdec{bi}")
            for g in range(2):
                cols = slice(64 * g, 64 * (g + 1))
                nc.scalar.activation(
                    dec[:, cols], tokreds[bi][:, cols],
                    mybir.ActivationFunctionType.Relu,
                )
                nc.tensor.matmul(
                    out_ps[:, cols],
                    c_t[:, bi * QPC:(bi + 1) * QPC],
                    dec[:, cols],
                    start=False, stop=(bi == NBLK - 1),
                    skip_group_check=True,
                )

        outsb = work.tile([QPC, BD], F32, tag="outsb")
        nc.scalar.copy(outsb[:], out_ps[:])
        nc.sync.dma_start(out_d[:], outsb[:])

    nc.compile()
    return nc


def build_nc(plan_name):
    if plan_name == "quad":
        return build_nc_quad()
    plan = PLANS[plan_name]
    ncol = sum(s[0] for s in sum(plan, []))
    assert sum(s[1] for s in sum(plan, [])) == BD
    for segs in plan:
        for ncols, ndocs, cap in segs:
            assert ncols == ndocs * cap

    nc = bacc.Bacc(
        "TRN2",
        target_bir_lowering=False,
        debug=False,
        num_devices=NCORES,
    )

    # A: [qlhsT (2 blocks x 128) | pruned rhs (ncol)] fp16
    a_d = nc.dram_tensor("a", [KC, NBLK * ROWS + ncol], F16, kind="ExternalInput")
    # B: [qclsT_hi (48) | qclsT_lo (48) | dclsT_hi (768)] fp16
    b_d = nc.dram_tensor("b", [128, 48 + CLS_D], F16, kind="ExternalInput")
    # C: per-token weight selectors, fp32 (paired with fp32 tokdec matmul)
    c_d = nc.dram_tensor("c", [128, NBLK * QPC], F16, kind="ExternalInput")
    out_d = nc.dram_tensor("out", [QPC, BD], F32, kind="ExternalOutput")

    rhs0 = NBLK * ROWS                     # rhs column origin inside A

    max_wave_banks = max(
        (sum(s[0] for s in segs) + TN - 1) // TN for segs in plan
    )
    psum_bufs = 3 if max_wave_banks <= 2 else 2

    with tile.TileContext(nc) as tc, ExitStack() as ctx:
        const = ctx.enter_context(tc.tile_pool(name="const", bufs=1))
        psum = ctx.enter_context(tc.tile_pool(name="psum", bufs=psum_bufs, space="PSUM"))
        opsum = ctx.enter_context(tc.tile_pool(name="opsum", bufs=1, space="PSUM"))
        work = ctx.enter_context(tc.tile_pool(name="work", bufs=1))

        a_t = const.tile([KC, NBLK * ROWS + ncol], F16, tag="a")
        b_t = const.tile([128, 48 + CLS_D], F16, tag="b")
        c_t = const.tile([128, NBLK * QPC], F16, tag="c")

        # Only sync + scalar have HW DGE queues.  DMA completion time is
        # governed by BYTES PER PARTITION ROW (rows move ~concurrently at
        # ~0.5 GB/s each), so split by COLUMNS into modest chunks issued in
        # consumption order, alternating engines.
        atot = NBLK * ROWS + ncol
        achunk = (atot + 3) // 4
        engs = [nc.sync, nc.scalar]
        for i in range(4):
            c0, c1 = i * achunk, min((i + 1) * achunk, atot)
            engs[i % 2].dma_start(a_t[:, c0:c1], a_d[:, c0:c1])
        nc.sync.dma_start(b_t[:, 0:408], b_d[:, 0:408])
        nc.scalar.dma_start(b_t[:, 408:], b_d[:, 408:])
        nc.scalar.dma_start(c_t[:], c_d[:])

        out_ps = opsum.tile([QPC, BD], F32, tag="out_ps")

        # score waves first (gated only by A); CLS (gated by B) after
        tokreds = []
        for bi in range(NBLK):
            lhs = a_t[:, bi * ROWS:(bi + 1) * ROWS]
            tokred = work.tile([ROWS, BD], F32, tag=f"tokred{bi}")
            tokreds.append(tokred)
            col = rhs0
            doc0 = 0
            for segs in plan:
                wcols = sum(s[0] for s in segs)
                ps = psum.tile([128, wcols], F32, tag="score")
                for k in range(0, wcols, TN):
                    n = min(TN, wcols - k)
                    nc.tensor.matmul(
                        ps[:, k:k + n], lhs, a_t[:, col + k:col + k + n],
                        start=True, stop=True,
                    )
                off = 0
                for ncols, ndocs, cap in segs:
                    red_in = ps[:, off:off + ncols].rearrange(
                        "p (d s) -> p d s", s=cap
                    )
                    nc.vector.reduce_max(
                        tokred[:, doc0:doc0 + ndocs],
                        red_in,
                        axis=mybir.AxisListType.X,
                    )
                    off += ncols
                    doc0 += ndocs
                col += wcols

        for k in range(6):
            nc.tensor.matmul(
                out_ps[:],
                b_t[:, k * QPC:(k + 1) * QPC],
                b_t[:, 48 + k * 128:48 + (k + 1) * 128],
                start=(k == 0), stop=False,
            )

        tokdec = []
        for bi in range(NBLK):
            dec = work.tile([ROWS, BD], F16, tag=f"tokdec{bi}")
            nc.scalar.activation(
                dec[:], tokreds[bi][:], mybir.ActivationFunctionType.Relu,
            )
            tokdec.append(dec)

        for bi in range(NBLK):
            nc.tensor.matmul(
                out_ps[:],
                c_t[:, bi * QPC:(bi + 1) * QPC],
                tokdec[bi][:],
                start=False, stop=(bi == NBLK - 1),
            )

        outsb = work.tile([QPC, BD], F32, tag="outsb")
        nc.scalar.copy(outsb[:], out_ps[:])
        nc.sync.dma_start(out_d[:], outsb[:])

    nc.compile()
    return nc


_NC_CACHE = {}


def _get_nc(plan_name):
    if plan_name not in _NC_CACHE:
        _NC_CACHE[plan_name] = build_nc(plan_name)
    return _NC_CACHE[plan_name]


def _bits_pm1(ids):
    """ids [...] int -> [..., NBITS] float32 of +/-1 binary-code digits."""
    ids = ids.astype(np.int64)
    shifts = np.arange(NBITS, dtype=np.int64)
    return ((ids[..., None] >> shifts) & 1).astype(np.float32) * 2.0 - 1.0


def _hilo16(x):
    hi = x.astype(np.float16)
    lo = (x - hi.astype(np.float32)).astype(np.float16)
    return hi, lo


def make_in_maps(qte, dte, qce, dce, qid, did, qam):
    # SEP mask + CLS drop -> per-token weights
    sep = qam.sum(1) - 1
    qm = qam.astype(np.float32).copy()
    qm[np.arange(BQ), sep] = 0.0
    w = qm.copy()
    w[:, 0] = 0.0

    # match-bonus scale C: must exceed any |score|; L2-norm bound, fp16-exact
    bound = float(
        np.linalg.norm(qte, axis=-1).max() * np.linalg.norm(dte, axis=-1).max()
    )
    C = 96.0
    while C <= bound * 1.1:
        C *= 2.0

    qbits = _bits_pm1(qid)                        # [64, 32, 13]
    dbits_all = _bits_pm1(did)                    # [128, 192, 13]
    dte16 = dte.astype(np.float16)

    # CLS blob (doc side shared, permuted per core below)
    dclsT_hi = np.ascontiguousarray(dce.T).astype(np.float16)   # [768, 128]

    # pick the smallest layout plan whose slot capacities fit every core
    percore_m = []
    pds = []
    for c in range(NCORES):
        cq = np.unique(qid[c * QPC:(c + 1) * QPC])
        m = np.isin(did, cq)
        percore_m.append(m)
        pds.append(np.sort(m.sum(1))[::-1])
    for plan_name in PLANS:
        if all(_plan_fits(plan_name, pd) for pd in pds):
            break
    if plan_name == "quad":
        plan_docs = [s for segs in QUAD_GROUP for s in segs]
    else:
        plan_docs = [s for segs in PLANS[plan_name] for s in segs]

    in_maps = []
    perms = []
    for c in range(NCORES):
        qs = slice(c * QPC, (c + 1) * QPC)
        qte_c, qbits_c, w_c = qte[qs], qbits[qs], w[qs]

        m = percore_m[c]
        # doc order: descending match count so the big-cap slots come first
        if plan_name == "dense":
            perm = np.arange(BD)
        else:
            perm = np.argsort(-m.sum(1), kind="stable")

        def fill_lhsT(a, p0):
            for bi in range(NBLK):
                blk = qte_c[bi * 4:(bi + 1) * 4].reshape(ROWS, TOK_D)
                cols = slice(bi * ROWS, (bi + 1) * ROWS)
                a[p0:p0 + TOK_D, cols] = blk.astype(np.float16).T
                a[p0 + TOK_D:p0 + TOK_D + NBITS, cols] = (
                    qbits_c[bi * 4:(bi + 1) * 4].reshape(ROWS, NBITS).T * C
                )
                a[p0 + KC - 1, cols] = 1.0

        def fill_doc(a, p0, col, d, cap):
            js = np.nonzero(m[d])[0] if cap < LD else np.arange(LD)
            e = col + len(js)
            a[p0:p0 + TOK_D, col:e] = dte16[d, js].T
            a[p0 + TOK_D:p0 + TOK_D + NBITS, col:e] = dbits_all[d, js].T
            a[p0 + KC - 1, col:e] = -NBITS * C

        if plan_name == "quad":
            # doc-slot column s = 64*g + gslot holds overall rank 2*gslot+g
            slot2doc = np.empty(BD, np.int64)
            gcols = sum(s[0] for s in plan_docs)
            a = np.zeros((110, NBLK * ROWS + gcols), np.float16)
            for g in range(2):
                fill_lhsT(a, 64 * g)
                col = NBLK * ROWS
                gslot = 0
                for ncols, ndocs, cap in plan_docs:
                    for k in range(ndocs):
                        d = perm[2 * gslot + g]
                        slot2doc[64 * g + gslot] = d
                        fill_doc(a, 64 * g, col, d, cap)
                        col += cap
                        gslot += 1
            perms.append(slot2doc)
        else:
            slot2doc = perm
            perms.append(slot2doc)
            ncol = sum(s[0] for s in plan_docs)
            a = np.zeros((KC, NBLK * ROWS + ncol), np.float16)
            fill_lhsT(a, 0)
            col = NBLK * ROWS
            di = 0
            for ncols, ndocs, cap in plan_docs:
                for k in range(ndocs):
                    fill_doc(a, 0, col, perm[di], cap)
                    col += cap
                    di += 1

        qclsT16 = qce[qs].T.astype(np.float16)    # [768, 8]
        b = np.zeros((128, 48 + CLS_D), np.float16)
        dperm = dclsT_hi[:, slot2doc]
        for k in range(6):
            ksl = slice(k * 128, (k + 1) * 128)
            b[:, k * QPC:(k + 1) * QPC] = qclsT16[ksl]
            b[:, 48 + k * 128:48 + (k + 1) * 128] = dperm[ksl]

        sel = np.zeros((128, NBLK * QPC), np.float16)
        for bi in range(NBLK):
            for qq in range(4):
                ql_ = bi * 4 + qq
                sel[qq * 32:(qq + 1) * 32, bi * QPC + ql_] = w_c[ql_]

        in_maps.append({"a": a, "b": b, "c": sel})
    return in_maps, plan_name, perms


def run(in_maps, plan_name="quad", trace=False, **kwargs):
    nc = _get_nc(plan_name)
    return run_bass_kernel_spmd(
        nc, in_maps, core_ids=list(range(NCORES)), trace=trace, **kwargs
    )


def kernel(
    query_tok_embs,
    doc_tok_embs,
    query_cls_emb,
    doc_cls_emb,
    query_input_ids,
    doc_input_ids,
    query_attention_mask,
):
    qte = np.ascontiguousarray(np.asarray(query_tok_embs, np.float32))
    dte = np.ascontiguousarray(np.asarray(doc_tok_embs, np.float32))
    qce = np.ascontiguousarray(np.asarray(query_cls_emb, np.float32))
    dce = np.ascontiguousarray(np.asarray(doc_cls_emb, np.float32))
    qid = np.asarray(query_input_ids).astype(np.int64)
    did = np.asarray(doc_input_ids).astype(np.int64)
    qam = np.asarray(query_attention_mask).astype(np.int64)

    in_maps, plan_name, perms = make_in_maps(qte, dte, qce, dce, qid, did, qam)
    res = run(in_maps, plan_name=plan_name)
    outs = []
    for c, r in enumerate(res.results):
        o = np.empty((QPC, BD), np.float32)
        o[:, perms[c]] = r["out"]
        outs.append(o)
    return np.ascontiguousarray(np.concatenate(outs, axis=0).astype(np.float32))


# revision 43
# speedup vs baseline: 1.0432x; 1.0432x over previous
"""COIL-style sparse-attention scoring kernel for Trainium2 (8 NeuronCores).

Reference computation:
    scores[q,i,d,j] = <query_tok_embs[q,i], doc_tok_embs[d,j]>         (K=32)
    masked = where(query_ids[q,i]==doc_ids[d,j], scores, 0)
    tok    = masked.max(axis=j)                                        (192 -> 1)
    tok_scores[q,d] = sum_i w[q,i] * tok[q,i,d]    (w drops CLS + SEP)
    out = tok_scores + query_cls_emb @ doc_cls_emb.T

Device strategy: data-parallel over the 64 queries (8 per core).  COIL is an
inverted-list model -- a doc position (d,j) can only contribute to a query
token with the *same* token id.  With |vocab|=5000 and 256 query tokens per
core, only ~5% of the 24576 doc positions match ANY of the core's query
tokens.  The host (integer id bookkeeping only -- all float math stays on
device) keeps just those columns, padded to a small per-doc slot budget
tiered by match count (docs sorted descending; see PLANS), shrinking the
score matmul from 24576 columns to ~900 per 128-row block.

Exact-match masking folds into the contraction via a 13-bit +/-1 id code:
  aug[i,col] = <q_i, d_col>                      (rows 0:32,  fp16 single term)
             + C * sum_b qbit[i,b]*dbit[col,b]   (rows 32:45, q side +/-C, doc +/-1)
             - 13*C                              (row 45: q side 1, doc -13C)
full id match    -> aug = score (code dot = +13C cancels the offset row)
id mismatch     -> aug <= score - 2C < 0   (C > max|score|, bounded on host)
padding columns -> aug = 0 (all-zero column)
so tok[i,d] = relu(max over the doc's slot columns) reproduces the reference
exactly: the reference max always sees a 0 from a non-matched where() zero,
because per-doc matches (<= 32) never cover all 192 positions.

The "quad" plan runs score matmuls in two 64-row PE quadrants (K=46 rows
duplicated at SBUF partitions 0:46 and 64:110, each quadrant owning half
the docs), which halves the bytes per SBUF partition row of the main blob
-- DMA completion time here is governed by bytes-per-row, not total bytes.
Per core: 12 fp16 score matmuls in 8 PSUM waves, DVE segmented max into
tok scores, ACT relu (fp16 out), CLS scores as 6 single-term fp16 matmuls
+ per-token weighted sums folded into the same [8,128] PSUM accumulation,
one output DMA.  All input DMAs ride the two hardware DGE queues
(sync/scalar) as column-chunks issued in consumption order; the software
gpsimd queue is never on the critical path.
"""

import numpy as np
from contextlib import ExitStack

import concourse.bass as bass
import concourse.bacc as bacc
import concourse.mybir as mybir
import concourse.tile as tile
from concourse.bass_utils import run_bass_kernel_spmd

F32 = mybir.dt.float32
F16 = mybir.dt.float16

# problem shape (hardcoded per contract)
BQ, LQ, BD, LD, TOK_D, CLS_D = 64, 32, 128, 192, 32, 768
NCORES = 8
QPC = BQ // NCORES          # 8 queries per core
NBLK = 2                    # two row-blocks of 128 = 4 queries x 32 tokens
ROWS = 128
NBITS = 13                  # 2^13 = 8192 > 5000 vocab
KC = TOK_D + NBITS + 1      # 46 = embs + id code + offset row
TN = 512                    # cols per matmul = one full PSUM bank


# wave plans: list of waves; each wave = list of (ncols, ndocs, cap) segments.
# docs are laid out in plan order (descending match count); tokred column
# offsets are implied.  Plans are tried in order; first whose slot
# capacities fit the core's match pattern wins.
#
# "quad" runs the score matmuls in two 64-row PE quadrants: the K=46
# contraction rows are duplicated at SBUF partitions 0:46 and 64:110, and
# each quadrant owns half the docs (alternating by rank).  That halves the
# bytes per SBUF partition row of the A blob, and DMA completion time is
# governed by bytes-per-row.  Group layout (per quadrant, per block):
# 16 docs @cap20 + 48 @cap12 = 896 columns, as an a-wave (1 PSUM bank,
# ready as soon as the first slice of A lands) and a b-wave (2 banks).
# A column order is [lhsT-b0 | seg-a | lhsT-b1 | seg-b] so the very first
# DMA chunk already carries everything the first sub-wave needs.
QUAD_GROUP = [[(320, 16, 20)], [(576, 48, 12)]]
QUAD_SUB0 = 4      # docs of the first a-wave computed as an early sub-wave
PLANS = {
    "quad": None,   # handled by the quadrant path; fit-checked like hetero2
    "hetero2": [[(256, 8, 32), (384, 24, 16)], [(768, 64, 12)], [(384, 32, 12)]],
    "hetero": [[(512, 16, 32), (512, 32, 16)], [(1024, 64, 16)], [(256, 16, 16)]],
    "uniform24": [[(1536, 64, 24)], [(1536, 64, 24)]],
    "dense": [[(1536, 8, 192)] for _ in range(16)],
}


def _plan_caps(plan_name):
    """per-rank slot capacity list (docs in descending match count)."""
    if plan_name == "quad":
        gcaps = []
        for ncols, ndocs, cap in (s for segs in QUAD_GROUP for s in segs):
            gcaps += [cap] * ndocs
        return [gcaps[r // 2] for r in range(BD)]
    caps = []
    for ncols, ndocs, cap in (s for segs in PLANS[plan_name] for s in segs):
        caps += [cap] * ndocs
    return caps


def _plan_fits(plan_name, pd_sorted):
    caps = _plan_caps(plan_name)
    return all(int(pd_sorted[r]) <= caps[r] for r in range(BD))


def build_nc_quad():
    gcols = sum(s[0] for s in sum(QUAD_GROUP, []))      # 896 per quadrant
    atot = NBLK * ROWS + gcols                          # 1152
    nc = bacc.Bacc(
        "TRN2",
        target_bir_lowering=False,
        debug=False,
        num_devices=NCORES,
    )
    a_d = nc.dram_tensor("a", [110, atot], F16, kind="ExternalInput")
    b_d = nc.dram_tensor("b", [128, 48 + CLS_D], F16, kind="ExternalInput")
    c_d = nc.dram_tensor("c", [128, NBLK * QPC], F16, kind="ExternalInput")
    out_d = nc.dram_tensor("out", [QPC, BD], F32, kind="ExternalOutput")

    rhs0 = NBLK * ROWS

    with tile.TileContext(nc) as tc, ExitStack() as ctx:
        const = ctx.enter_context(tc.tile_pool(name="const", bufs=1))
        # a-waves (320 cols = 1 bank) and b-waves (576 cols = 2 banks) get
        # separate pools: 3*1 + 2*2 + 1 output bank = 8 PSUM banks exactly
        pa = ctx.enter_context(tc.tile_pool(name="pa", bufs=3, space="PSUM"))
        pb = ctx.enter_context(tc.tile_pool(name="pb", bufs=2, space="PSUM"))
        opsum = ctx.enter_context(tc.tile_pool(name="opsum", bufs=1, space="PSUM"))
        work = ctx.enter_context(tc.tile_pool(name="work", bufs=1))

        a_t = const.tile([110, atot], F16, tag="a")
        b_t = const.tile([128, 48 + CLS_D], F16, tag="b")
        c_t = const.tile([128, NBLK * QPC], F16, tag="c")

        # A column map: [lhsT-b0 (128) | seg-a | lhsT-b1 (128) | seg-b]
        acols = sum(s[0] for s in QUAD_GROUP[0])          # 320
        bcols = sum(s[0] for s in QUAD_GROUP[1])          # 576
        (na, nda, capa), = QUAD_GROUP[0]
        lhs_off = [0, ROWS + acols]
        aoff = ROWS                                        # seg-a origin
        boff = 2 * ROWS + acols                            # seg-b origin
        sub0 = QUAD_SUB0 * capa                            # first sub-wave cols

        engs = [nc.sync, nc.scalar]
        # chunk edges chosen so: c0 = lhsT-b0 + first sub-wave, c1 = rest of
        # seg-a, c2 = lhsT-b1, c3/c4 = seg-b halves; alternating engines
        edges = [0, ROWS + sub0, ROWS + acols, boff,
                 boff + bcols // 2, atot]
        for i in range(5):
            c0, c1 = edges[i], edges[i + 1]
            engs[i % 2].dma_start(a_t[:, c0:c1], a_d[:, c0:c1])
        nc.sync.dma_start(b_t[:, 0:408], b_d[:, 0:408])
        nc.scalar.dma_start(b_t[:, 408:], b_d[:, 408:])
        nc.scalar.dma_start(c_t[:], c_d[:])

        out_ps = opsum.tile([QPC, BD], F32, tag="out_ps")

        tokreds = []
        for bi in range(NBLK):
            tokreds.append(
                work.tile([ROWS, BD], F32, tag=f"tokred{bi}", name=f"tokred{bi}")
            )

        def emit_wave(pool, p0, bi, cols0, ncols, slot, cap, tag, grp):
            lhs = a_t[p0:p0 + KC, lhs_off[bi]:lhs_off[bi] + ROWS]
            ps = pool.tile([128, ncols], F32, tag=grp, name=tag)
            for k in range(0, ncols, TN):
                n = min(TN, ncols - k)
                nc.tensor.matmul(
                    ps[:, k:k + n], lhs,
                    a_t[p0:p0 + KC, cols0 + k:cols0 + k + n],
                    start=True, stop=True,
                )
            nc.vector.reduce_max(
                tokreds[bi][:, slot:slot + ncols // cap],
                ps[:, :].rearrange("p (d s) -> p d s", s=cap),
                axis=mybir.AxisListType.X,
            )

        with tc.high_priority():
            # a-waves (seg-a): first one split so the DVE chain starts as
            # soon as the very first DMA chunk lands
            for bi in range(NBLK):
                for g in range(2):
                    slot = 64 * g
                    if bi == 0 and g == 0:
                        emit_wave(pa, 0, 0, aoff, sub0, slot, capa, "ps_a00s", "sa")
                        emit_wave(pa, 0, 0, aoff + sub0, acols - sub0,
                                  slot + QUAD_SUB0, capa, "ps_a00r", "sa")
                    else:
                        emit_wave(pa, 64 * g, bi, aoff, acols, slot, capa,
                                  f"ps_a{bi}{g}", "sa")
            # b-waves (seg-b)
            (nb, ndb, capb), = QUAD_GROUP[1]
            for bi in range(NBLK):
                for g in range(2):
                    emit_wave(pb, 64 * g, bi, boff, bcols, 64 * g + nda,
                              capb, f"ps_b{bi}{g}", "sb")

        for k in range(6):
            nc.tensor.matmul(
                out_ps[:],
                b_t[:, k * QPC:(k + 1) * QPC],
                b_t[:, 48 + k * 128:48 + (k + 1) * 128],
                start=(k == 0), stop=False,
            )

        # relu + weighted sum per (block, quadrant) piece: each [*, 64]
        # column region finishes as soon as its reduces land.  fp16 dec and
        # sel weights (exact 0/1) make these 1-cycle/col matmuls.  The very
        # last piece runs its relu on the DVE itself: same-engine chaining
        # after the final reduce skips a cross-engine semaphore hop.
        for bi in range(NBLK):
            dec = work.tile([ROWS, BD], F16, tag=f"tokdec{bi}")
            for g in range(2):
                cols = slice(64 * g, 64 * (g + 1))
                if bi == NBLK - 1 and g == 1:
                    nc.vector.tensor_scalar_max(
                        dec[:, cols], tokreds[bi][:, cols], 0.0
                    )
                else:
                    nc.scalar.activation(
                        dec[:, cols], tokreds[bi][:, cols],
                        mybir.ActivationFunctionType.Relu,
                    )
                nc.tensor.matmul(
                    out_ps[:, cols],
                    c_t[:, bi * QPC:(bi + 1) * QPC],
                    dec[:, cols],
                    start=False, stop=(bi == NBLK - 1),
                    skip_group_check=True,
                )

        outsb = work.tile([QPC, BD], F32, tag="outsb")
        nc.scalar.copy(outsb[:], out_ps[:])
        nc.sync.dma_start(out_d[:], outsb[:])

    nc.compile()
    return nc


def build_nc(plan_name):
    if plan_name == "quad":
        return build_nc_quad()
    plan = PLANS[plan_name]
    ncol = sum(s[0] for s in sum(plan, []))
    assert sum(s[1] for s in sum(plan, [])) == BD
    for segs in plan:
        for ncols, ndocs, cap in segs:
            assert ncols == ndocs * cap

    nc = bacc.Bacc(
        "TRN2",
        target_bir_lowering=False,
        debug=False,
        num_devices=NCORES,
    )

    # A: [qlhsT (2 blocks x 128) | pruned rhs (ncol)] fp16
    a_d = nc.dram_tensor("a", [KC, NBLK * ROWS + ncol], F16, kind="ExternalInput")
    # B: [qclsT_hi (48) | qclsT_lo (48) | dclsT_hi (768)] fp16
    b_d = nc.dram_tensor("b", [128, 48 + CLS_D], F16, kind="ExternalInput")
    # C: per-token weight selectors, fp32 (paired with fp32 tokdec matmul)
    c_d = nc.dram_tensor("c", [128, NBLK * QPC], F16, kind="ExternalInput")
    out_d = nc.dram_tensor("out", [QPC, BD], F32, kind="ExternalOutput")

    rhs0 = NBLK * ROWS                     # rhs column origin inside A

    max_wave_banks = max(
        (sum(s[0] for s in segs) + TN - 1) // TN for segs in plan
    )
    psum_bufs = 3 if max_wave_banks <= 2 else 2

    with tile.TileContext(nc) as tc, ExitStack() as ctx:
        const = ctx.enter_context(tc.tile_pool(name="const", bufs=1))
        psum = ctx.enter_context(tc.tile_pool(name="psum", bufs=psum_bufs, space="PSUM"))
        opsum = ctx.enter_context(tc.tile_pool(name="opsum", bufs=1, space="PSUM"))
        work = ctx.enter_context(tc.tile_pool(name="work", bufs=1))

        a_t = const.tile([KC, NBLK * ROWS + ncol], F16, tag="a")
        b_t = const.tile([128, 48 + CLS_D], F16, tag="b")
        c_t = const.tile([128, NBLK * QPC], F16, tag="c")

        # Only sync + scalar have HW DGE queues.  DMA completion time is
        # governed by BYTES PER PARTITION ROW (rows move ~concurrently at
        # ~0.5 GB/s each), so split by COLUMNS into modest chunks issued in
        # consumption order, alternating engines.
        atot = NBLK * ROWS + ncol
        achunk = (atot + 3) // 4
        engs = [nc.sync, nc.scalar]
        for i in range(4):
            c0, c1 = i * achunk, min((i + 1) * achunk, atot)
            engs[i % 2].dma_start(a_t[:, c0:c1], a_d[:, c0:c1])
        nc.sync.dma_start(b_t[:, 0:408], b_d[:, 0:408])
        nc.scalar.dma_start(b_t[:, 408:], b_d[:, 408:])
        nc.scalar.dma_start(c_t[:], c_d[:])

        out_ps = opsum.tile([QPC, BD], F32, tag="out_ps")

        # score waves first (gated only by A); CLS (gated by B) after
        tokreds = []
        for bi in range(NBLK):
            lhs = a_t[:, bi * ROWS:(bi + 1) * ROWS]
            tokred = work.tile([ROWS, BD], F32, tag=f"tokred{bi}")
            tokreds.append(tokred)
            col = rhs0
            doc0 = 0
            for segs in plan:
                wcols = sum(s[0] for s in segs)
                ps = psum.tile([128, wcols], F32, tag="score")
                for k in range(0, wcols, TN):
                    n = min(TN, wcols - k)
                    nc.tensor.matmul(
                        ps[:, k:k + n], lhs, a_t[:, col + k:col + k + n],
                        start=True, stop=True,
                    )
                off = 0
                for ncols, ndocs, cap in segs:
                    red_in = ps[:, off:off + ncols].rearrange(
                        "p (d s) -> p d s", s=cap
                    )
                    nc.vector.reduce_max(
                        tokred[:, doc0:doc0 + ndocs],
                        red_in,
                        axis=mybir.AxisListType.X,
                    )
                    off += ncols
                    doc0 += ndocs
                col += wcols

        for k in range(6):
            nc.tensor.matmul(
                out_ps[:],
                b_t[:, k * QPC:(k + 1) * QPC],
                b_t[:, 48 + k * 128:48 + (k + 1) * 128],
                start=(k == 0), stop=False,
            )

        tokdec = []
        for bi in range(NBLK):
            dec = work.tile([ROWS, BD], F16, tag=f"tokdec{bi}")
            nc.scalar.activation(
                dec[:], tokreds[bi][:], mybir.ActivationFunctionType.Relu,
            )
            tokdec.append(dec)

        for bi in range(NBLK):
            nc.tensor.matmul(
                out_ps[:],
                c_t[:, bi * QPC:(bi + 1) * QPC],
                tokdec[bi][:],
                start=False, stop=(bi == NBLK - 1),
            )

        outsb = work.tile([QPC, BD], F32, tag="outsb")
        nc.scalar.copy(outsb[:], out_ps[:])
        nc.sync.dma_start(out_d[:], outsb[:])

    nc.compile()
    return nc


_NC_CACHE = {}


def _get_nc(plan_name):
    if plan_name not in _NC_CACHE:
        _NC_CACHE[plan_name] = build_nc(plan_name)
    return _NC_CACHE[plan_name]


def _bits_pm1(ids):
    """ids [...] int -> [..., NBITS] float32 of +/-1 binary-code digits."""
    ids = ids.astype(np.int64)
    shifts = np.arange(NBITS, dtype=np.int64)
    return ((ids[..., None] >> shifts) & 1).astype(np.float32) * 2.0 - 1.0


def make_in_maps(qte, dte, qce, dce, qid, did, qam):
    # SEP mask + CLS drop -> per-token weights
    sep = qam.sum(1) - 1
    qm = qam.astype(np.float32).copy()
    qm[np.arange(BQ), sep] = 0.0
    w = qm.copy()
    w[:, 0] = 0.0

    # match-bonus scale C: must exceed any |score|; L2-norm bound, fp16-exact
    bound = float(
        np.linalg.norm(qte, axis=-1).max() * np.linalg.norm(dte, axis=-1).max()
    )
    C = 96.0
    while C <= bound * 1.1:
        C *= 2.0

    qbits = _bits_pm1(qid)                        # [64, 32, 13]
    dbits_all = _bits_pm1(did)                    # [128, 192, 13]
    dte16 = dte.astype(np.float16)

    # CLS blob (doc side shared, permuted per core below)
    dclsT_hi = np.ascontiguousarray(dce.T).astype(np.float16)   # [768, 128]

    # pick the smallest layout plan whose slot capacities fit every core
    percore_m = []
    pds = []
    for c in range(NCORES):
        cq = np.unique(qid[c * QPC:(c + 1) * QPC])
        m = np.isin(did, cq)
        percore_m.append(m)
        pds.append(np.sort(m.sum(1))[::-1])
    for plan_name in PLANS:
        if all(_plan_fits(plan_name, pd) for pd in pds):
            break
    if plan_name == "quad":
        plan_docs = [s for segs in QUAD_GROUP for s in segs]
    else:
        plan_docs = [s for segs in PLANS[plan_name] for s in segs]

    in_maps = []
    perms = []
    for c in range(NCORES):
        qs = slice(c * QPC, (c + 1) * QPC)
        qte_c, qbits_c, w_c = qte[qs], qbits[qs], w[qs]

        m = percore_m[c]
        # doc order: descending match count so the big-cap slots come first
        if plan_name == "dense":
            perm = np.arange(BD)
        else:
            perm = np.argsort(-m.sum(1), kind="stable")

        def fill_lhsT(a, p0, col_off=None):
            for bi in range(NBLK):
                blk = qte_c[bi * 4:(bi + 1) * 4].reshape(ROWS, TOK_D)
                c0 = bi * ROWS if col_off is None else col_off[bi]
                cols = slice(c0, c0 + ROWS)
                a[p0:p0 + TOK_D, cols] = blk.astype(np.float16).T
                a[p0 + TOK_D:p0 + TOK_D + NBITS, cols] = (
                    qbits_c[bi * 4:(bi + 1) * 4].reshape(ROWS, NBITS).T * C
                )
                a[p0 + KC - 1, cols] = 1.0

        def fill_doc(a, p0, col, d, cap):
            js = np.nonzero(m[d])[0] if cap < LD else np.arange(LD)
            e = col + len(js)
            a[p0:p0 + TOK_D, col:e] = dte16[d, js].T
            a[p0 + TOK_D:p0 + TOK_D + NBITS, col:e] = dbits_all[d, js].T
            a[p0 + KC - 1, col:e] = -NBITS * C

        if plan_name == "quad":
            # doc-slot column s = 64*g + gslot holds overall rank 2*gslot+g
            # A column order: [lhsT-b0 | seg-a | lhsT-b1 | seg-b]
            slot2doc = np.empty(BD, np.int64)
            acols = sum(s[0] for s in QUAD_GROUP[0])
            gcols = sum(s[0] for s in plan_docs)
            a = np.zeros((110, NBLK * ROWS + gcols), np.float16)
            seg_col0 = [ROWS, 2 * ROWS + acols]
            for g in range(2):
                fill_lhsT(a, 64 * g, col_off=[0, ROWS + acols])
                gslot = 0
                for si, (ncols, ndocs, cap) in enumerate(plan_docs):
                    col = seg_col0[si]
                    for k in range(ndocs):
                        d = perm[2 * gslot + g]
                        slot2doc[64 * g + gslot] = d
                        fill_doc(a, 64 * g, col, d, cap)
                        col += cap
                        gslot += 1
            perms.append(slot2doc)
        else:
            slot2doc = perm
            perms.append(slot2doc)
            ncol = sum(s[0] for s in plan_docs)
            a = np.zeros((KC, NBLK * ROWS + ncol), np.float16)
            fill_lhsT(a, 0)
            col = NBLK * ROWS
            di = 0
            for ncols, ndocs, cap in plan_docs:
                for k in range(ndocs):
                    fill_doc(a, 0, col, perm[di], cap)
                    col += cap
                    di += 1

        qclsT16 = qce[qs].T.astype(np.float16)    # [768, 8]
        b = np.zeros((128, 48 + CLS_D), np.float16)
        dperm = dclsT_hi[:, slot2doc]
        for k in range(6):
            ksl = slice(k * 128, (k + 1) * 128)
            b[:, k * QPC:(k + 1) * QPC] = qclsT16[ksl]
            b[:, 48 + k * 128:48 + (k + 1) * 128] = dperm[ksl]

        sel = np.zeros((128, NBLK * QPC), np.float16)
        for bi in range(NBLK):
            for qq in range(4):
                ql_ = bi * 4 + qq
                sel[qq * 32:(qq + 1) * 32, bi * QPC + ql_] = w_c[ql_]

        in_maps.append({"a": a, "b": b, "c": sel})
    return in_maps, plan_name, perms


def run(in_maps, plan_name="quad", trace=False, **kwargs):
    nc = _get_nc(plan_name)
    return run_bass_kernel_spmd(
        nc, in_maps, core_ids=list(range(NCORES)), trace=trace, **kwargs
    )


def kernel(
    query_tok_embs,
    doc_tok_embs,
    query_cls_emb,
    doc_cls_emb,
    query_input_ids,
    doc_input_ids,
    query_attention_mask,
):
    qte = np.ascontiguousarray(np.asarray(query_tok_embs, np.float32))
    dte = np.ascontiguousarray(np.asarray(doc_tok_embs, np.float32))
    qce = np.ascontiguousarray(np.asarray(query_cls_emb, np.float32))
    dce = np.ascontiguousarray(np.asarray(doc_cls_emb, np.float32))
    qid = np.asarray(query_input_ids).astype(np.int64)
    did = np.asarray(doc_input_ids).astype(np.int64)
    qam = np.asarray(query_attention_mask).astype(np.int64)

    in_maps, plan_name, perms = make_in_maps(qte, dte, qce, dce, qid, did, qam)
    res = run(in_maps, plan_name=plan_name)
    outs = []
    for c, r in enumerate(res.results):
        o = np.empty((QPC, BD), np.float32)
        o[:, perms[c]] = r["out"]
        outs.append(o)
    return np.ascontiguousarray(np.concatenate(outs, axis=0).astype(np.float32))
